# revision 25
# baseline (speedup 1.0000x reference)
"""Trainium2 Bass kernel: AdditiveAttention-style scoring head.

Computes, for x:(B,N,D), W1/W2:(A,D), b1/b2:(A,), Wout:(A,), bout:(1,):
    x1 = x @ W1.T + b1                       (B,N,A)
    x2 = x @ W2.T + b2                       (B,N,A)
    out[b,i-1,j] = sum_a Wout[a]*tanh(x1[b,j,a] + x2[b,i,a]) + bout,  i=1..N-1

Sharding: data-parallel over batch B across 8 NeuronCores (B/8=4 per core),
weights replicated, no collectives.

Algorithm: approximate tanh(s) by a 5-term Fourier sine series
tanh(s) ~= sum_k c_k sin(k*w0*s), which separates per harmonic:
    sin(k*w0*(u+v)) = sin(k*w0*u)cos(k*w0*v) + cos(k*w0*u)sin(k*w0*v)
so the (N,N,A) tanh contraction becomes 2K rank-A matmul chains per batch
on the PE (one PSUM bank per batch: matmul start zeroes the whole bank).
ACT evaluates the seeds sin(w0*z+w0*b) and sin(w0/2*z+..) straight out of
the input-GEMM PSUM (args stay inside the HW sin table's [-pi,pi] range by
construction of w0); cos via half-angle, harmonics 2..5 via product
identities (sin3x = sinx(2cos2x+1) etc.) and Chebyshev steps.

HW lessons baked in (measured on device): DVE tensor_scalar needs AP
(pointer) scalars — immediate scalars take a ~30us first-use hit; DVE
tensor_tensor runs 2x on packed bf16 (0.6ns/col); GpSimd is 6-40x slower
than DVE on elementwise and cannot access PSUM, so it only drives DMA
queues; scalar_tensor_tensor lowers to a slow path (avoided); DMA cannot
read PSUM, so outputs stage through ACT Identity; both sides share one
[128, 4096] tile per function so each elementwise op covers the whole
problem in one instruction; dummy matmuls with staggered tile deps sit
between scoring k-groups to hold the PE HAM clock at 8/8.
"""
import sys
import numpy as np

if "/opt/trn_rl_repo" not in sys.path:
    sys.path.insert(0, "/opt/trn_rl_repo")

B, N, D, A = 32, 128, 512, 512
NCORES = 8
BPC = B // NCORES      # batches per core
TOK = BPC * N          # tokens per core
KC = D // 128          # contraction chunks for the input matmuls
MC = A // 128          # a-chunks
FK = 5                 # Fourier harmonics
HC = MC * TOK          # 2048: one side's columns in a paired tile
W0 = float(np.pi / (2 * 3.2) * 0.995)
COEF = [1.1989471, -0.0654593, 0.26222026, -0.04736725, 0.06483877]

_CACHE = {}


def _build_nc():
    import concourse.bass as bass
    import concourse.bacc as bacc
    import concourse.mybir as mybir
    from concourse import tile

    f32 = mybir.dt.float32
    bf16 = mybir.dt.bfloat16
    AF = mybir.ActivationFunctionType
    OP = mybir.AluOpType

    nc = bacc.Bacc(None, target_bir_lowering=False)

    xT = nc.declare_dram_parameter("xT", [D, TOK], bf16, isOutput=False)
    # w?t[m, d, j] = W?[m*128+j, d] — a-chunk-major
    w1t = nc.declare_dram_parameter("w1t", [MC, D, 128], bf16, isOutput=False)
    w2t = nc.declare_dram_parameter("w2t", [MC, D, 128], bf16, isOutput=False)
    # bvec?[p, c] = W0*b?[c*128+p]; bvec?[p, MC+c] = 0.5*W0*b?[c*128+p]
    bvec1 = nc.declare_dram_parameter("bvec1", [128, 2 * MC], f32, isOutput=False)
    bvec2 = nc.declare_dram_parameter("bvec2", [128, 2 * MC], f32, isOutput=False)
    # wcvec[p, (k-1)*MC + c] = Wout[c*128+p] * COEF[k-1]
    wcvec_d = nc.declare_dram_parameter("wcvec", [128, MC * FK], f32, isOutput=False)
    boutr = nc.declare_dram_parameter("boutr", [1, 128], bf16, isOutput=False)
    out = nc.declare_dram_parameter("out", [BPC, (N - 1) * N], f32, isOutput=True)

    with tile.TileContext(nc) as tc:
        with (
            tc.tile_pool(name="const", bufs=1) as cpool,
            tc.tile_pool(name="xw", bufs=1) as xwpool,
            tc.tile_pool(name="f", bufs=1) as fpool,
            tc.tile_pool(name="sc", bufs=5) as spool,
            tc.tile_pool(name="stage", bufs=4) as stpool,
        ):
            # ---- consts + PE warmup on junk data during the DMA window ----
            warm = cpool.tile([128, 512], bf16, tag="warm")
            nc.vector.memset(warm[:, :], 0.25)
            ones = cpool.tile([1, 128], bf16, tag="ones")
            nc.vector.memset(ones[:, :], 1.0)
            k2v = cpool.tile([128, 1], f32, tag="k2v")     # 2.0
            nc.vector.memset(k2v[:, :], 2.0)
            k1v = cpool.tile([128, 1], f32, tag="k1v")     # 1.0
            nc.vector.memset(k1v[:, :], 1.0)
            km2v = cpool.tile([128, 1], f32, tag="km2v")   # -2.0
            nc.vector.memset(km2v[:, :], -2.0)
            with tc.tile_pool(name="psW", bufs=1, space=bass.MemorySpace.PSUM) as psW:
                wps = psW.tile([128, 512], f32, tag="psW")
                for _ in range(9):
                    nc.tensor.matmul(wps[:, :], warm[:, 0:128], warm[:, :],
                                     start=True, stop=True)

            # ---- input DMAs ----
            xT_sb = []
            for k in range(KC):
                tx = xwpool.tile([128, TOK], bf16, tag=f"xT{k}")
                eng = nc.sync if k % 2 == 0 else nc.scalar
                eng.dma_start(tx[:, :], xT[k * 128:(k + 1) * 128, :])
                xT_sb.append(tx)
            w1_sb, w2_sb = [], []
            for m in range(MC):
                t2 = xwpool.tile([128, KC * 128], bf16, tag=f"w2{m}", name=f"w2_{m}")
                d2 = t2[:, :]
                dst2 = bass.AP(d2.tensor, d2.offset,
                               [[d2.ap[0][0], 128], [128, KC], [1, 128]])
                src2 = bass.AP(w2t[0, :, :].tensor, m * D * 128,
                               [[128, 128], [128 * 128, KC], [1, 128]])
                nc.gpsimd.dma_start(dst2, src2)
                w2_sb.append(t2)
                t1 = xwpool.tile([128, KC * 128], bf16, tag=f"w1{m}", name=f"w1_{m}")
                d1 = t1[:, :]
                dst1 = bass.AP(d1.tensor, d1.offset,
                               [[d1.ap[0][0], 128], [128, KC], [1, 128]])
                src1 = bass.AP(w1t[0, :, :].tensor, m * D * 128,
                               [[128, 128], [128 * 128, KC], [1, 128]])
                nc.scalar.dma_start(dst1, src1)
                w1_sb.append(t1)
            bv1 = cpool.tile([128, 2 * MC], f32, tag="bv1")
            nc.sync.dma_start(bv1[:, :], bvec1[:, :])
            bv2 = cpool.tile([128, 2 * MC], f32, tag="bv2")
            nc.sync.dma_start(bv2[:, :], bvec2[:, :])
            wcv = cpool.tile([128, MC * FK], f32, tag="wcv")
            nc.sync.dma_start(wcv[:, :], wcvec_d[:, :])
            boutt = cpool.tile([1, 128], bf16, tag="boutt")
            nc.sync.dma_start(boutt[:, :], boutr[:, :])

            # ---- paired function tiles [128, 2*HC]: cols 0..HC-1 = side 1
            # (x2, lhsT source), cols HC.. = side 0 (x1, rhs source).
            # Within a side: col = c*TOK + b*N + t. ----
            def ftile(nm):
                return fpool.tile([128, 2 * HC], bf16, tag=nm, name=nm)
            S1, SH, Q1, QH, Q2 = (ftile("s1"), ftile("sh"), ftile("q1"),
                                  ftile("qh"), ftile("q2"))
            CC, T0, T2, T2M = ftile("cc"), ftile("t0"), ftile("t2"), ftile("t2m")
            C1, C2, S2, S3, C3, S4, C4, S5, C5 = (
                ftile("c1"), ftile("c2"), ftile("s2"), ftile("s3"), ftile("c3"),
                ftile("s4"), ftile("c4"), ftile("s5"), ftile("c5"))
            SF = [None, S1, S2, S3, S4, S5]
            CF = [None, C1, C2, C3, C4, C5]

            with (
                tc.tile_pool(name="psG", bufs=3, space=bass.MemorySpace.PSUM) as psG,
                tc.tile_pool(name="psO", bufs=4, space=bass.MemorySpace.PSUM) as psO,
                tc.tile_pool(name="psT", bufs=1, space=bass.MemorySpace.PSUM) as psT,
            ):
                tps = psT.tile([128, 128], f32, tag="psT")

                def keepwarm(dep_ap):
                    nc.tensor.matmul(tps[:, :], dep_ap, warm[:, 0:128],
                                     start=True, stop=True)

                # ---- input GEMMs; ACT consumes PSUM directly. side 1 (x2)
                # fills cols [0, HC), side 0 (x1) fills [HC, 2*HC). ----
                for side, w_sb, bv, base in ((1, w2_sb, bv2, 0), (0, w1_sb, bv1, HC)):
                    for c in range(MC):
                        ps = psG.tile([128, TOK], f32, tag="psG", name=f"g{side}_{c}")
                        for k in range(KC):
                            nc.tensor.matmul(ps[:, :], w_sb[c][:, k * 128:(k + 1) * 128],
                                             xT_sb[k][:, :],
                                             start=(k == 0), stop=(k == KC - 1))
                        sl = slice(base + c * TOK, base + (c + 1) * TOK)
                        nc.scalar.activation(S1[:, sl], ps[:, :], AF.Sin,
                                             bias=bv[:, c:c + 1], scale=W0)
                        nc.scalar.activation(SH[:, sl], ps[:, :], AF.Sin,
                                             bias=bv[:, MC + c:MC + c + 1],
                                             scale=0.5 * W0)
                    # squares for this side right away: the downstream chain
                    # (c1 -> sc1 -> first scoring matmuls) hangs off them
                    hs = slice(base, base + HC)
                    nc.scalar.activation(QH[:, hs], SH[:, hs], AF.Square)
                    nc.scalar.activation(Q1[:, hs], S1[:, hs], AF.Square)

                # ---- per-side-half elementwise, side 1 (x2) first so the
                # scoring stationaries materialize early; the two sides'
                # chains interleave on the DVE queue so dependency latency
                # overlaps. Squares on ACT right after each side's seeds. ----
                v = nc.vector
                HA = [slice(0, HC), slice(HC, 2 * HC)]   # HA[0]=x2, HA[1]=x1
                scq = {}   # scaling tiles per (k, t)

                def emit_scalings(k):
                    for t in range(2):
                        src = CF[k] if t == 0 else SF[k]
                        sc = spool.tile([128, HC], bf16, tag="sc", name=f"sc{k}_{t}")
                        for c in range(MC):
                            v.tensor_scalar(sc[:, c * TOK:(c + 1) * TOK],
                                            src[:, c * TOK:(c + 1) * TOK],
                                            wcv[:, (k - 1) * MC + c:(k - 1) * MC + c + 1],
                                            None, OP.mult)
                        scq[(k, t)] = sc

                # DVE stream in dependency-priority order: the x2-side (HA[0])
                # ops and each k's scalings come before the matching x1-side
                # ops, so every scoring k-group unblocks as early as possible.
                x2, x1 = HA[0], HA[1]

                def ts(dst, src, a, s1v, s2v, op0, op1=None):
                    if op1 is None:
                        v.tensor_scalar(dst[:, a], src[:, a], s1v, None, op0)
                    else:
                        v.tensor_scalar(dst[:, a], src[:, a], s1v, s2v, op0, op1)

                def tt(dst, in0, in1, a, op):
                    v.tensor_tensor(dst[:, a], in0[:, a], in1[:, a], op)

                ts(C1, QH, x2, km2v[:, 0:1], k1v[:, 0:1], OP.mult, OP.add)
                emit_scalings(1)
                ts(C1, QH, x1, km2v[:, 0:1], k1v[:, 0:1], OP.mult, OP.add)
                ts(CC, C1, x2, k2v[:, 0:1], None, OP.mult)
                tt(S2, S1, CC, x2, OP.mult)
                ts(C2, Q1, x2, km2v[:, 0:1], k1v[:, 0:1], OP.mult, OP.add)
                ts(CC, C1, x1, k2v[:, 0:1], None, OP.mult)
                tt(S2, S1, CC, x1, OP.mult)
                ts(C2, Q1, x1, km2v[:, 0:1], k1v[:, 0:1], OP.mult, OP.add)
                emit_scalings(2)
                nc.scalar.activation(Q2[:, x2], S2[:, x2], AF.Square)
                nc.scalar.activation(Q2[:, x1], S2[:, x1], AF.Square)
                ts(T0, C2, x2, k2v[:, 0:1], None, OP.mult)
                ts(T2, T0, x2, k1v[:, 0:1], None, OP.add)
                ts(T2M, T0, x2, k1v[:, 0:1], None, OP.subtract)
                tt(S3, S1, T2, x2, OP.mult)
                tt(C3, C1, T2M, x2, OP.mult)
                emit_scalings(3)
                tt(S4, S2, T0, x2, OP.mult)
                ts(T0, C2, x1, k2v[:, 0:1], None, OP.mult)
                ts(T2, T0, x1, k1v[:, 0:1], None, OP.add)
                ts(T2M, T0, x1, k1v[:, 0:1], None, OP.subtract)
                tt(S3, S1, T2, x1, OP.mult)
                tt(C3, C1, T2M, x1, OP.mult)
                tt(S4, S2, T0, x1, OP.mult)
                ts(C4, Q2, x2, km2v[:, 0:1], k1v[:, 0:1], OP.mult, OP.add)
                emit_scalings(4)
                ts(C4, Q2, x1, km2v[:, 0:1], k1v[:, 0:1], OP.mult, OP.add)
                tt(S5, CC, S4, x2, OP.mult)
                tt(S5, S5, S3, x2, OP.subtract)
                tt(C5, CC, C4, x2, OP.mult)
                tt(C5, C5, C3, x2, OP.subtract)
                emit_scalings(5)
                tt(S5, CC, S4, x1, OP.mult)
                tt(S5, S5, S3, x1, OP.subtract)
                tt(C5, CC, C4, x1, OP.mult)
                tt(C5, C5, C3, x1, OP.subtract)

                # ---- scoring: 32 matmuls per (k, term) accumulating
                # psm_b[i, j]; keepwarm matmuls with staggered deps bridge
                # the PE gaps so the HAM stays at 8/8 ----
                psm = [psO.tile([128, 128], f32, tag="psO", name=f"psm{b}")
                       for b in range(BPC)]
                kwdep = {2: S3, 3: S4, 4: S5, 5: C5}
                kwdep2 = {2: C2, 3: C3, 4: C4, 5: S5}
                keepwarm(S2[:, 0:128])
                keepwarm(S2[:, HC:HC + 128])
                for k in range(1, FK + 1):
                    if k in kwdep:
                        keepwarm(kwdep2[k][:, 0:128])
                        keepwarm(kwdep[k][:, 0:128])
                    for t in range(2):
                        sc = scq[(k, t)]
                        rhs = SF[k] if t == 0 else CF[k]
                        for c in range(MC):
                            for b in range(BPC):
                                lo = c * TOK + b * N
                                nc.tensor.matmul(psm[b][:, :],
                                                 sc[:, lo:lo + N],
                                                 rhs[:, HC + lo:HC + lo + N],
                                                 start=(k == 1 and t == 0 and c == 0),
                                                 stop=False)
                # bout injection (rank-1: boutt^T @ ones) + chain stop
                for b in range(BPC):
                    nc.tensor.matmul(psm[b][:, :], boutt[:, :], ones[:, :],
                                     start=False, stop=True)

                # ---- stage PSUM->SBUF on ACT, DMA out on 3 queues ----
                oap = out[:, :]
                qeng = [nc.sync, nc.scalar, nc.sync, nc.scalar]
                for b in range(BPC):
                    stg = stpool.tile([128, 128], f32, tag="stg", name=f"stg{b}")
                    nc.scalar.activation(stg[:, :], psm[b][:, :], AF.Identity)
                    dst = bass.AP(oap.tensor, oap.offset + b * (N - 1) * N,
                                  [[N, N - 1], [1, N]])
                    qeng[b].dma_start(dst, stg[1:128, :])

    nc.finalize()
    return nc


def _get_nc():
    if "nc" not in _CACHE:
        _CACHE["nc"] = _build_nc()
    return _CACHE["nc"]


def _prep_in_maps(x, W1, b1, W2, b2, Wout, bout):
    import ml_dtypes
    f = np.float32
    bf = ml_dtypes.bfloat16
    w1t = np.ascontiguousarray(
        np.asarray(W1, f).reshape(MC, 128, D).transpose(0, 2, 1).astype(bf))
    w2t = np.ascontiguousarray(
        np.asarray(W2, f).reshape(MC, 128, D).transpose(0, 2, 1).astype(bf))
    b1c = np.asarray(b1, f).reshape(MC, 128).T   # [128, MC]
    b2c = np.asarray(b2, f).reshape(MC, 128).T
    b1v = np.concatenate([W0 * b1c, 0.5 * W0 * b1c], axis=1)
    b2v = np.concatenate([W0 * b2c, 0.5 * W0 * b2c], axis=1)
    Wo = np.asarray(Wout, f).reshape(MC, 128).T  # [128, MC]
    wcv = np.empty((128, MC * FK), f)
    for k in range(FK):
        wcv[:, k * MC:(k + 1) * MC] = Wo * COEF[k]
    bor = np.full((1, 128), np.asarray(bout, f).reshape(()), f).astype(bf)
    x = np.asarray(x, f)
    in_maps = []
    for ci in range(NCORES):
        xs = x[ci * BPC:(ci + 1) * BPC]
        xTi = np.ascontiguousarray(
            xs.transpose(2, 0, 1).reshape(D, TOK).astype(bf))
        in_maps.append({
            "xT": xTi, "w1t": w1t, "w2t": w2t,
            "bvec1": np.ascontiguousarray(b1v),
            "bvec2": np.ascontiguousarray(b2v),
            "wcvec": wcv, "boutr": bor,
        })
    return in_maps


def _run(x, W1, b1, W2, b2, Wout, bout, trace=False):
    from concourse.bass_utils import run_bass_kernel_spmd

    nc = _get_nc()
    in_maps = _prep_in_maps(x, W1, b1, W2, b2, Wout, bout)
    res = run_bass_kernel_spmd(nc, in_maps, core_ids=list(range(NCORES)), trace=trace)
    outs = [np.asarray(res.results[ci]["out"]).reshape(BPC, N - 1, N)
            for ci in range(NCORES)]
    full = np.concatenate(outs, axis=0).astype(np.float32)
    return full, res


def kernel(x, W1, b1, W2, b2, Wout, bout):
    full, _ = _run(x, W1, b1, W2, b2, Wout, bout, trace=False)
    return full
